# revision 39
# baseline (speedup 1.0000x reference)
"""TRN2 Bass kernel for DeepAveragingLSTMNetwork (8 NeuronCores, SPMD).

Strategy (data-parallel over words per the sharding hint, plus a ragged
schedule and an fp8 DoubleRow recurrence):
  * Words with char_length < 2 contribute nothing to the char-LSTM pooled
    vector (reference zeroes them) -> excluded from LSTM shards.
  * Remaining words are sorted by length (desc) and dealt round-robin to
    8 cores, padded per length-level with dummy columns so every core has
    the IDENTICAL length profile; the per-step active count m_t is then a
    compile-time schedule shared by all cores.  Dummy columns carry an
    all-zero one-hot AND a zero bias row, so their h stays exactly 0 and
    no pooling mask is needed.
  * LSTM in transposed layout: state h^T [H, words] as fp8e4m3, cell c^T
    bf16.  Per (gate,chunk) PSUM tile the contraction is 3 matmuls:
      1 bf16  : Gaug^T-as-lhsT @ onehot_t   (Gaug = [char_embed@W_ih^T ; b])
      2 fp8 DR: DoubleRow matmuls, K=256 each, covering W_hh^T (K=512)
    The bias is folded into the one-hot matmul via a constant ones row, so
    the sigmoid of gates i,f,o fuses into ONE activation instruction over
    3 PSUM banks.
  * PSUM: per chunk one [128, 4gates, 512] tile (4 banks), double-buffered.
    MM issue order interleaves chunk groups so the recurrent phase-1 MMs
    (which need the freshest h chunks) issue as late as possible, and each
    chunk's activations run while the next chunk's matmuls stream.
  * Cell math on DVE in bf16 (2x mode) as per-chunk chains that start at
    each chunk's own sigmoid; the trailing chunk of each pair releases its
    i,f sigmoid slice before o so the chain starts ~m cycles earlier.
  * Small steps (m <= 256) pack two chunks into one [128, 2, 4, 256] PSUM
    tile; only the even gate slot of each shared 2KB bank carries
    start=True (PSUM pending-zero is bank-granular).
  * Retiring (frozen) columns are reduced into the pooled accumulator
    immediately (no final mask/reduce pass).
  * glove half: host gathers each core's 512 word rows (index re-encoding,
    like the one-hot) transposed to [128, 3, 512]; one DVE reduce makes the
    partition-major partial sum.  No 100k-row table stream, no histogram.
  * Collectives: a 1-float dummy AllReduce up front absorbs the one-time
    global barrier; the glove AllReduce runs mid-LSTM; anchored dummy
    AllReduces keep the cc stack warm/cores synced so the tail char
    AllReduce is short.  A small bf16 head runs replicated on every core;
    the 1/N mean folds into the fc1 sigmoid's scale argument.
"""

import sys
import time

for _p in ("/opt/trn_rl_repo",):
    if _p not in sys.path:
        sys.path.append(_p)

import numpy as np
import ml_dtypes

import concourse.bass as bass
import concourse.bacc as bacc
import concourse.mybir as mybir
import concourse.tile as tile
from concourse.bass_utils import run_bass_kernel_spmd

NCORES = 8
F32 = mybir.dt.float32
F32R = mybir.dt.float32r
BF16 = mybir.dt.bfloat16
FP8 = mybir.dt.float8e4
DR = mybir.MatmulPerfMode.DoubleRow


def _build_shards(char_lengths, L):
    """Index-only host prep: per-core word lists ((-1) = dummy), the shared
    schedule m_t, and per-core masks."""
    lengths = np.asarray(char_lengths)
    keep = np.where(lengths >= 2)[0]
    order = keep[np.argsort(-lengths[keep], kind="stable")]
    lens_sorted = lengths[order]

    core_pos = [[] for _ in range(NCORES)]
    core_mask = [[] for _ in range(NCORES)]
    profile = []
    idx = 0
    for l in range(L, 1, -1):
        c = int((lens_sorted == l).sum())
        if c == 0:
            continue
        n_l = (c + NCORES - 1) // NCORES
        words = order[idx:idx + c]
        idx += c
        for ci in range(NCORES):
            take = words[ci::NCORES]
            for w in take:
                core_pos[ci].append(int(w))
                core_mask[ci].append(1.0)
            for _ in range(n_l - len(take)):
                core_pos[ci].append(-1)
                core_mask[ci].append(0.0)
        profile.extend([l] * n_l)
    profile = np.array(profile)
    m_t = [int((profile > t).sum()) for t in range(L)]
    n = len(profile)
    return core_pos, core_mask, m_t, n


def _build_program(n, m_t, L, VC, DC, H, DW, HID, OUT, gwords, n_total, skip=()):
    H4 = 4 * H
    KH = H // 128                       # 4 h chunks
    GK = (128 * ((DW + 127) // 128)) // 128   # glove partition-major cols (3)
    SP = KH + GK                        # fused partial width (7)
    steps = [t for t in range(L) if m_t[t] > 0]
    KMLP = SP                           # head K-tiles

    nc = bacc.Bacc(num_devices=NCORES)

    oh_ext = nc.declare_dram_parameter("onehot", [len(steps), 128, n], BF16, isOutput=False)
    gath_ext = nc.declare_dram_parameter("gath", [128, GK, gwords], F32, isOutput=False)
    whh8_ext = nc.declare_dram_parameter("whh8", [128, KH, H4], FP8, isOutput=False)
    wih_ext = nc.declare_dram_parameter("wihT", [128, H4], BF16, isOutput=False)
    cemb_ext = nc.declare_dram_parameter("cembT", [128, VC], BF16, isOutput=False)
    bias_ext = nc.declare_dram_parameter("bias", [1, H4], BF16, isOutput=False)
    fc1_ext = nc.declare_dram_parameter("fc1T", [128, KMLP, HID], BF16, isOutput=False)
    fc1b_ext = nc.declare_dram_parameter("fc1b", [128, HID // 128], F32, isOutput=False)
    fc2_ext = nc.declare_dram_parameter("fc2T", [128, HID // 128, OUT], BF16, isOutput=False)
    fc2b_ext = nc.declare_dram_parameter("fc2b", [OUT], F32, isOutput=False)
    out_ext = nc.declare_dram_parameter("out", [1, OUT], F32, isOutput=True)

    sc_part = nc.dram_tensor("sc_part", [128 * KH], F32)
    sc_red = nc.dram_tensor("sc_red", [128 * KH], F32, addr_space="Shared")
    sg_part = nc.dram_tensor("sg_part", [128 * GK], F32)
    sg_red = nc.dram_tensor("sg_red", [128 * GK], F32, addr_space="Shared")
    du_part = nc.dram_tensor("du_part", [1], F32)
    du_red = nc.dram_tensor("du_red", [1], F32, addr_space="Shared")

    Sig = mybir.ActivationFunctionType.Sigmoid
    Tanh = mybir.ActivationFunctionType.Tanh
    Copy = mybir.ActivationFunctionType.Copy
    AX = mybir.AxisListType.X
    ADD = mybir.AluOpType.add
    MUL = mybir.AluOpType.mult

    with tile.TileContext(nc) as tc:
        with (
            tc.tile_pool(name="consts", bufs=1) as consts,
            tc.tile_pool(name="ohp", bufs=4) as ohp,
            tc.tile_pool(name="cell", bufs=2) as cell,
            tc.tile_pool(name="psg", bufs=2, space="PSUM") as psg,
        ):
            # dummy 1-float collective up front: absorbs the one-time global
            # collective BARRIER (~30us) under the LSTM instead of at the tail
            if "coll" not in skip:
                nc.gpsimd.collective_compute(
                    "AllReduce", ADD,
                    replica_groups=[list(range(NCORES))],
                    ins=[du_part[:]], outs=[du_red[:]],
                )

            # ---- critical-path constants (G feeds step 0) ----
            cemb_sb = consts.tile([128, VC], BF16, tag="cemb_sb")
            nc.sync.dma_start(out=cemb_sb, in_=cemb_ext[:, :])
            wih_sb = consts.tile([128, H4], BF16, tag="wih_sb")
            nc.sync.dma_start(out=wih_sb, in_=wih_ext[:, :])

            # Gaug = [char_embed @ W_ih^T ; bias row at VC; zeros above]
            g_bf = consts.tile([128, H4], BF16, tag="g_bf")
            nc.vector.memset(g_bf, 0.0)
            nc.sync.dma_start(out=g_bf[VC:VC + 1, :], in_=bias_ext[:, :])
            for c in range(H4 // 512):
                g_ps = psg.tile([128, 4, 512], F32, tag="ps")
                nc.tensor.matmul(
                    g_ps[:VC, 0, :],
                    cemb_sb,
                    wih_sb[:, c * 512:(c + 1) * 512],
                    start=True, stop=True,
                )
                nc.scalar.activation(g_bf[:VC, c * 512:(c + 1) * 512], g_ps[:VC, 0, :], Copy)

            # prefetch the first onehot slabs BEFORE the 1.6MB weight streams
            # so step 0 starts as soon as G is ready
            oh_pre = []
            for pi in range(2):
                ohp_t = ohp.tile([128, n], BF16, tag="oh", name=f"oh_pre{pi}")
                nc.sync.dma_start(out=ohp_t[:, :m_t[pi]], in_=oh_ext[pi, :, :m_t[pi]])
                oh_pre.append(ohp_t)
            whh8 = consts.tile([128, KH, H4], FP8, tag="whh8")
            nc.sync.dma_start(out=whh8, in_=whh8_ext[:, :, :])
            gath_sb = consts.tile([128, GK, gwords], F32, tag="gath_sb")
            nc.sync.dma_start(out=gath_sb, in_=gath_ext[:, :, :])

            one_sb = consts.tile([128, 1], F32, tag="one_sb")
            nc.vector.memset(one_sb, 1.0)

            # fused pooled-partial tile: [:, 0:KH] char sum, [:, KH:SP] glove
            sp_sb = consts.tile([128, SP], F32, tag="sp_sb")
            nc.vector.memset(sp_sb[:, 0:KH], 0.0)

            # ---- LSTM state, split per DR plane-pair so dependency tracking
            # stays precise: phase-0 matmuls wait only on h planes 0,1 ----
            hA = [consts.tile([128, 2, n], FP8, tag=f"hA{i}", name=f"hA{i}")
                  for i in range(2)]
            hB = [consts.tile([128, 2, n], FP8, tag=f"hB{i}", name=f"hB{i}")
                  for i in range(2)]
            cA = consts.tile([128, 2, n], BF16, tag="cA")
            cB = consts.tile([128, 2, n], BF16, tag="cB")

            def xmm(ps, oh_sb, j, m, stop):
                # one-hot (+bias row) matmuls: slot order g,i,f,o = gates 2,0,1,3
                for s, gate in enumerate((2, 0, 1, 3)):
                    mm = gate * KH + j
                    nc.tensor.matmul(
                        ps[:, s, :m],
                        g_bf[:, mm * 128:(mm + 1) * 128],
                        oh_sb[:, :m],
                        start=True, stop=stop,
                    )

            def rmm(ps, hp, m, j, stop):
                # fp8 DoubleRow recurrent matmuls over one h plane-pair tile
                p = 0 if hp in hA else 1
                for s, gate in enumerate((2, 0, 1, 3)):
                    mm = gate * KH + j
                    nc.tensor.matmul(
                        ps[:, s, :m],
                        whh8[:, 2 * p:2 * p + 2, mm * 128:(mm + 1) * 128],
                        hp[:, :, :m],
                        perf_mode=DR,
                        start=False, stop=stop,
                    )

            def acts(ps, ifo, gg, q, m, split=False):
                # gate activations for one chunk; sigmoid over i,f,o fused.
                # split=True releases i,f first so the cell chain (which only
                # needs i,f,g) starts ~m cycles earlier; o follows for h.
                nc.scalar.activation(gg[:, q, :m], ps[:, 0, :m], Tanh)
                if split:
                    nc.scalar.activation(ifo[:, q, 0:2, :m], ps[:, 1:3, :m], Sig)
                    nc.scalar.activation(ifo[:, q, 2, :m], ps[:, 3, :m], Sig)
                else:
                    nc.scalar.activation(ifo[:, q, :, :m], ps[:, 1:4, :m], Sig)

            def xmm2(ps, oh_sb, q, j, m, stop):
                # one-hot matmuls into slot q of a two-chunk pair tile.
                # PSUM start=True zeroes a whole 2KB bank; with two 1KB gate
                # slots per bank only the even slot may carry start, the odd
                # slot accumulates onto the pending-zeroed bytes.
                for s, gate in enumerate((2, 0, 1, 3)):
                    mm = gate * KH + j
                    nc.tensor.matmul(
                        ps[:, q, s, :m],
                        g_bf[:, mm * 128:(mm + 1) * 128],
                        oh_sb[:, :m],
                        start=(s % 2 == 0), stop=stop,
                        skip_group_check=True,
                    )

            def rmm2(ps, hp, m, q, j, stop):
                p = 0 if hp in hA else 1
                for s, gate in enumerate((2, 0, 1, 3)):
                    mm = gate * KH + j
                    nc.tensor.matmul(
                        ps[:, q, s, :m],
                        whh8[:, 2 * p:2 * p + 2, mm * 128:(mm + 1) * 128],
                        hp[:, :, :m],
                        perf_mode=DR,
                        start=False, stop=stop,
                        skip_group_check=True,
                    )

            def acts_pair(ps, ifo, gg, m):
                # gate activations for both chunks of a pair in 2 instructions
                nc.scalar.activation(gg[:, :, :m], ps[:, :, 0, :m], Tanh)
                nc.scalar.activation(ifo[:, :, :, :m], ps[:, :, 1:4, :m], Sig)

            def dve_c(ifo, gg, ig, cX, q, m, first):
                # c update for one chunk (slot q of its pair tiles); the
                # f*c multiply runs on the otherwise-idle gpsimd engine,
                # concurrent with i*g on DVE
                nc.vector.tensor_tensor(ig[:, q, :m], ifo[:, q, 0, :m],
                                        gg[:, q, :m], op=MUL)
                if first:
                    nc.vector.tensor_copy(cX[:, q, :m], ig[:, q, :m])
                else:
                    nc.gpsimd.tensor_tensor(cX[:, q, :m], ifo[:, q, 1, :m],
                                            cX[:, q, :m], op=MUL)
                    nc.vector.tensor_tensor(cX[:, q, :m], cX[:, q, :m],
                                            ig[:, q, :m], op=ADD)

            def dve_h(ifo, tc_sb, cX, hX_wr, q, m):
                # tanh(c) then h = o * tanh(c), one chunk
                nc.scalar.activation(tc_sb[:, q, :m], cX[:, q, :m], Tanh)
                nc.vector.tensor_tensor(hX_wr[:, q, :m],
                                        ifo[:, q, 2, :m], tc_sb[:, q, :m], op=MUL)

            for si, t in enumerate(steps):
                m = m_t[t]
                hA_rd, hA_wr = hA[si % 2], hA[(si + 1) % 2]
                hB_rd, hB_wr = hB[si % 2], hB[(si + 1) % 2]
                if si < 2:
                    oh_sb = oh_pre[si]
                else:
                    oh_sb = ohp.tile([128, n], BF16, tag="oh")
                    nc.sync.dma_start(out=oh_sb[:, :m], in_=oh_ext[si, :, :m])

                ifoA = cell.tile([128, 2, 3, n], BF16, tag="ifoA")
                ggA = cell.tile([128, 2, n], BF16, tag="ggA")
                igA = cell.tile([128, 2, n], BF16, tag="igA")
                tcA = cell.tile([128, 2, n], BF16, tag="tcA")
                ifoB = cell.tile([128, 2, 3, n], BF16, tag="ifoB")
                ggB = cell.tile([128, 2, n], BF16, tag="ggB")
                igB = cell.tile([128, 2, n], BF16, tag="igB")
                tcB = cell.tile([128, 2, n], BF16, tag="tcB")

                first = si == 0
                if m > 256:
                    # PE order: x0 p00 x1 p01 p10 p11 x2 p02 x3 p03 p12 p13.
                    # x-MMs open each group (no fresh deps), phase-1 MMs are
                    # the stop legs; per-chunk cell chains start at each
                    # chunk's own sigmoid so pair A's h lands before the next
                    # step's p0 matmuls and pair B's before its p1 matmuls.
                    ps0 = psg.tile([128, 4, 512], F32, tag="ps", name="ps0")
                    xmm(ps0, oh_sb, 0, m, stop=first)
                    ps1 = psg.tile([128, 4, 512], F32, tag="ps", name="ps1")
                    xmm(ps1, oh_sb, 1, m, stop=first)
                    if not first:
                        rmm(ps0, hA_rd, m, 0, stop=False)
                        rmm(ps1, hA_rd, m, 1, stop=False)
                        rmm(ps0, hB_rd, m, 0, stop=True)
                    acts(ps0, ifoA, ggA, 0, m)
                    dve_c(ifoA, ggA, igA, cA, 0, m, first)
                    if not first:
                        rmm(ps1, hB_rd, m, 1, stop=True)
                    acts(ps1, ifoA, ggA, 1, m, split=True)
                    dve_c(ifoA, ggA, igA, cA, 1, m, first)
                    dve_h(ifoA, tcA, cA, hA_wr, 0, m)
                    dve_h(ifoA, tcA, cA, hA_wr, 1, m)
                    ps2 = psg.tile([128, 4, 512], F32, tag="ps", name="ps2")
                    xmm(ps2, oh_sb, 2, m, stop=first)
                    ps3 = psg.tile([128, 4, 512], F32, tag="ps", name="ps3")
                    xmm(ps3, oh_sb, 3, m, stop=first)
                    if not first:
                        rmm(ps2, hA_rd, m, 2, stop=False)
                        rmm(ps3, hA_rd, m, 3, stop=False)
                        rmm(ps2, hB_rd, m, 2, stop=True)
                    acts(ps2, ifoB, ggB, 0, m)
                    dve_c(ifoB, ggB, igB, cB, 0, m, first)
                    if not first:
                        rmm(ps3, hB_rd, m, 3, stop=True)
                    acts(ps3, ifoB, ggB, 1, m, split=True)
                    dve_c(ifoB, ggB, igB, cB, 1, m, first)
                    dve_h(ifoB, tcB, cB, hB_wr, 0, m)
                    dve_h(ifoB, tcB, cB, hB_wr, 1, m)
                else:
                    # small steps: both chunks of a pair share one PSUM tile
                    # [128, 8, 256] (slots q*4+s), halving ACT instruction
                    # count (tg/sig/tanh-c fire once per pair) and group count
                    psA = psg.tile([128, 2, 4, 256], F32, tag="ps", name="psA")
                    for q in (0, 1):
                        xmm2(psA, oh_sb, q, 0 * 2 + q, m, stop=first)
                    psB = psg.tile([128, 2, 4, 256], F32, tag="ps", name="psB")
                    for q in (0, 1):
                        xmm2(psB, oh_sb, q, 1 * 2 + q, m, stop=first)
                    if not first:
                        for q in (0, 1):
                            rmm2(psA, hA_rd, m, q, 0 * 2 + q, stop=False)
                        for q in (0, 1):
                            rmm2(psA, hB_rd, m, q, 0 * 2 + q, stop=True)
                    acts_pair(psA, ifoA, ggA, m)
                    dve_c(ifoA, ggA, igA, cA, 0, m, first)
                    dve_c(ifoA, ggA, igA, cA, 1, m, first)
                    dve_h(ifoA, tcA, cA, hA_wr, 0, m)
                    dve_h(ifoA, tcA, cA, hA_wr, 1, m)
                    if not first:
                        for q in (0, 1):
                            rmm2(psB, hA_rd, m, q, 1 * 2 + q, stop=False)
                        for q in (0, 1):
                            rmm2(psB, hB_rd, m, q, 1 * 2 + q, stop=True)
                    acts_pair(psB, ifoB, ggB, m)
                    dve_c(ifoB, ggB, igB, cB, 0, m, first)
                    dve_c(ifoB, ggB, igB, cB, 1, m, first)
                    dve_h(ifoB, tcB, cB, hB_wr, 0, m)
                    dve_h(ifoB, tcB, cB, hB_wr, 1, m)

                if si == 2:
                    # glove partial: one reduce over the gathered columns,
                    # then its AllReduce mid-LSTM (also re-syncs the cores so
                    # the tail char AllReduce runs at warm-collective speed)
                    nc.vector.tensor_reduce(sp_sb[:, KH:SP], gath_sb, axis=AX, op=ADD)
                    sg_pm = sg_part.rearrange("(p k) -> p k", k=GK)
                    nc.sync.dma_start(out=sg_pm, in_=sp_sb[:, KH:SP])
                    if "coll" in skip:
                        nc.sync.dma_start(out=sg_red[:], in_=sg_part[:])
                    else:
                        nc.gpsimd.collective_compute(
                            "AllReduce", ADD,
                            replica_groups=[list(range(NCORES))],
                            ins=[sg_part[:]], outs=[sg_red[:]],
                        )
                if si in (10, 16) and "coll" not in skip:
                    # keep the collective stack warm + cores synced so the
                    # tail AllReduce doesn't pay a cold/skew penalty; the DMA
                    # from this step's state anchors the collective in time
                    nc.sync.dma_start(out=du_part[:], in_=sp_sb[0:1, 0])
                    nc.gpsimd.collective_compute(
                        "AllReduce", ADD,
                        replica_groups=[list(range(NCORES))],
                        ins=[du_part[:]], outs=[du_red[:]],
                    )

                next_m = m_t[steps[si + 1]] if si + 1 < len(steps) else 0
                if next_m < m:
                    rt = cell.tile([128, KH], F32, tag="rt")
                    nc.vector.tensor_reduce(rt[:, 0:2], hA_wr[:, :, next_m:m], axis=AX, op=ADD)
                    nc.vector.tensor_reduce(rt[:, 2:4], hB_wr[:, :, next_m:m], axis=AX, op=ADD)
                    nc.vector.tensor_tensor(sp_sb[:, 0:KH], sp_sb[:, 0:KH], rt, op=ADD)

                if si == len(steps) - 6:
                    # head weights mid-kernel so the DMA queue is clear early
                    fc1_sb = consts.tile([128, KMLP, HID], BF16, tag="fc1_sb")
                    nc.sync.dma_start(out=fc1_sb, in_=fc1_ext[:, :, :])
                    fc1b_sb = consts.tile([128, HID // 128], F32, tag="fc1b_sb")
                    nc.sync.dma_start(out=fc1b_sb, in_=fc1b_ext[:, :])
                    fc2_sb = consts.tile([128, HID // 128, OUT], BF16, tag="fc2_sb")
                    nc.sync.dma_start(out=fc2_sb, in_=fc2_ext[:, :, :])
                    fc2b_sb = consts.tile([128, 1], F32, tag="fc2b_sb")
                    nc.sync.dma_start(out=fc2b_sb[:OUT, 0], in_=fc2b_ext[:])

            # ---- char partial -> tail AllReduce ----
            sc_pm = sc_part.rearrange("(p k) -> p k", k=KH)
            nc.sync.dma_start(out=sc_pm, in_=sp_sb[:, 0:KH])
            if "coll" in skip:
                nc.sync.dma_start(out=sc_red[:], in_=sc_part[:])
            else:
                nc.gpsimd.collective_compute(
                    "AllReduce", ADD,
                    replica_groups=[list(range(NCORES))],
                    ins=[sc_part[:]], outs=[sc_red[:]],
                )

            # ---- head MLP (identical on every core; bf16 matmuls) ----
            avg_f = consts.tile([128, SP], F32, tag="avg_f")
            nc.sync.dma_start(out=avg_f[:, 0:KH],
                              in_=sc_red.rearrange("(p k) -> p k", k=KH))
            nc.sync.dma_start(out=avg_f[:, KH:SP],
                              in_=sg_red.rearrange("(p k) -> p k", k=GK))
            avg_sb = consts.tile([128, SP], BF16, tag="avg_sb")
            nc.vector.tensor_copy(avg_sb, avg_f)
            # fc1 preacts directly partition-major: accumulate K=1 matvecs
            # per 128-wide hidden chunk (no transpose pass needed)
            pc_ps = psg.tile([128, 4, 512], F32, tag="ps", name="pc_ps")
            for i in range(HID // 128):
                for k in range(KMLP):
                    nc.tensor.matmul(pc_ps[:, 0, i:i + 1],
                                     fc1_sb[:, k, i * 128:(i + 1) * 128],
                                     avg_sb[:, k:k + 1],
                                     start=(k == 0), stop=(k == KMLP - 1))
            h1_sb = consts.tile([128, HID // 128], BF16, tag="h1_sb")
            for i in range(HID // 128):
                nc.scalar.activation(h1_sb[:, i:i + 1], pc_ps[:, 0, i:i + 1], Sig,
                                     bias=fc1b_sb[:, i:i + 1], scale=1.0 / n_total)
            lo_sb = consts.tile([128, 1], F32, tag="lo_sb")
            lp = psg.tile([128, 4, 512], F32, tag="ps", name="lp")
            for k in range(HID // 128):
                nc.tensor.matmul(lp[:OUT, 0, 0:1], fc2_sb[:, k, :], h1_sb[:, k:k + 1],
                                 start=(k == 0), stop=(k == HID // 128 - 1))
            nc.vector.tensor_tensor(lo_sb[:OUT, :], lp[:OUT, 0, 0:1], fc2b_sb[:OUT, :], op=ADD)
            nc.sync.dma_start(out=out_ext[0, :], in_=lo_sb[:OUT, 0])

    nc.compile()
    return nc


def kernel(**inputs):
    word_indices = np.asarray(inputs["word_indices"])
    char_indices = np.asarray(inputs["char_indices"])
    char_lengths = np.asarray(inputs["char_lengths"])
    glove_table = np.ascontiguousarray(np.asarray(inputs["glove_table"], dtype=np.float32))
    char_embed = np.asarray(inputs["char_embed"], dtype=np.float32)
    W_ih = np.asarray(inputs["W_ih"], dtype=np.float32)
    W_hh = np.asarray(inputs["W_hh"], dtype=np.float32)
    b_ih = np.asarray(inputs["b_ih"], dtype=np.float32)
    b_hh = np.asarray(inputs["b_hh"], dtype=np.float32)
    fc1_W = np.asarray(inputs["fc1_W"], dtype=np.float32)
    fc1_b = np.asarray(inputs["fc1_b"], dtype=np.float32)
    fc2_W = np.asarray(inputs["fc2_W"], dtype=np.float32)
    fc2_b = np.asarray(inputs["fc2_b"], dtype=np.float32)

    N, L = char_indices.shape
    VW, DW = glove_table.shape
    VC, DC = char_embed.shape
    H = W_hh.shape[1]
    H4 = 4 * H
    HID = fc1_W.shape[0]
    OUT = fc2_W.shape[0]
    KH = H // 128

    core_pos, core_mask, m_t, n = _build_shards(char_lengths, L)
    assert n <= 512, f"per-core shard {n} exceeds one PSUM bank"
    steps = [t for t in range(L) if m_t[t] > 0]
    gwords = N // NCORES

    nc = _build_program(n, m_t, L, VC, DC, H, DW, HID, OUT, gwords, N)

    GK = (128 * ((DW + 127) // 128)) // 128
    SP = KH + GK
    # fc1 rows: char block first [H], then glove block [DW], padded to 128*SP
    fc1T = np.zeros((128 * SP, HID), np.float32)
    fc1T[:H] = fc1_W[:, DW:].T
    fc1T[H:H + DW] = fc1_W[:, :DW].T
    shared = dict(
        whh8=np.ascontiguousarray(
            W_hh.T.reshape(KH, 128, H4).transpose(1, 0, 2)).astype(ml_dtypes.float8_e4m3fn),
        wihT=np.ascontiguousarray(W_ih.T).astype(ml_dtypes.bfloat16),
        cembT=np.ascontiguousarray(char_embed.T).astype(ml_dtypes.bfloat16),
        bias=(b_ih + b_hh).reshape(1, H4).astype(ml_dtypes.bfloat16),
        fc1T=np.ascontiguousarray(
            fc1T.reshape(SP, 128, HID).transpose(1, 0, 2)).astype(ml_dtypes.bfloat16),
        fc1b=np.ascontiguousarray(fc1_b.reshape(-1, 128).T),
        fc2T=np.ascontiguousarray(
            fc2_W.T.reshape(-1, 128, OUT).transpose(1, 0, 2)).astype(ml_dtypes.bfloat16),
        fc2b=fc2_b,
    )

    in_maps = []
    for ci in range(NCORES):
        pos = core_pos[ci]
        mask = np.array(core_mask[ci], np.float32)
        # one-hot char encodings [steps, 128, n] with bias row VC = validity
        oh = np.zeros((len(steps), 128, n), ml_dtypes.bfloat16)
        widx = np.array([w if w >= 0 else 0 for w in pos])
        ci_shard = char_indices[widx]                    # [n, L]
        real = mask > 0
        cols = np.arange(n)
        for si, t in enumerate(steps):
            oh[si, ci_shard[real, t], cols[real]] = 1.0
            oh[si, VC, real] = 1.0
        # gathered glove rows, transposed partition-major [128, GK, gwords]
        words = word_indices[ci * gwords:(ci + 1) * gwords]
        g = np.zeros((128 * GK, gwords), np.float32)
        g[:DW] = glove_table[words].T
        in_maps.append(dict(
            onehot=oh,
            gath=np.ascontiguousarray(g.reshape(GK, 128, gwords).transpose(1, 0, 2)),
            **shared,
        ))

    # the axon/NRT stack occasionally reports a transient device error
    # (NRT_EXEC_UNIT_UNRECOVERABLE); a retry on fresh state recovers it
    res = None
    for attempt in range(3):
        try:
            res = run_bass_kernel_spmd(nc, in_maps, list(range(NCORES)))
            break
        except Exception:
            if attempt == 2:
                raise
            time.sleep(2.0)
    global _LAST_RESULTS
    _LAST_RESULTS = res
    return np.array(res.results[0]["out"], dtype=np.float32)


_LAST_RESULTS = None


# revision 43
# speedup vs baseline: 1.1316x; 1.1316x over previous
"""TRN2 Bass kernel for DeepAveragingLSTMNetwork (8 NeuronCores, SPMD).

Strategy (data-parallel over words per the sharding hint, plus a ragged
schedule and an fp8 DoubleRow recurrence):
  * Words with char_length < 2 contribute nothing to the char-LSTM pooled
    vector (reference zeroes them) -> excluded from LSTM shards.
  * Remaining words are sorted by length (desc) and dealt round-robin to
    8 cores, padded per length-level with dummy columns so every core has
    the IDENTICAL length profile; the per-step active count m_t is then a
    compile-time schedule shared by all cores.  Dummy columns carry an
    all-zero one-hot AND a zero bias row, so their h stays exactly 0 and
    no pooling mask is needed.
  * LSTM in transposed layout: state h^T [H, words] as fp8e4m3, cell c^T
    bf16.  Per (gate,chunk) PSUM tile the contraction is 3 matmuls:
      1 bf16  : Gaug^T-as-lhsT @ onehot_t   (Gaug = [char_embed@W_ih^T ; b])
      2 fp8 DR: DoubleRow matmuls, K=256 each, covering W_hh^T (K=512)
    The bias is folded into the one-hot matmul via a constant ones row, so
    the sigmoid of gates i,f,o fuses into ONE activation instruction over
    3 PSUM banks.
  * PSUM: per chunk one [128, 4gates, 512] tile (4 banks), double-buffered.
    MM issue order interleaves chunk groups so the recurrent phase-1 MMs
    (which need the freshest h chunks) issue as late as possible, and each
    chunk's activations run while the next chunk's matmuls stream.
  * Cell math on DVE in bf16 (2x mode), processed in half-step pairs of
    chunks to halve instruction overhead.
  * Retiring (frozen) columns are reduced into the pooled accumulator
    immediately (no final mask/reduce pass).
  * glove half: host gathers each core's 512 word rows (index re-encoding,
    like the one-hot) transposed to [128, 3, 512]; one DVE reduce makes the
    partition-major partial sum.  No 100k-row table stream, no histogram.
  * ONE fused AllReduce (char 512 + glove 384 dims) and a small bf16/f32r
    head replicated on every core; the 1/N mean folds into the sigmoid's
    scale argument.
"""

import sys
import time

for _p in ("/opt/trn_rl_repo",):
    if _p not in sys.path:
        sys.path.append(_p)

import numpy as np
import ml_dtypes

import concourse.bass as bass
import concourse.bacc as bacc
import concourse.mybir as mybir
import concourse.tile as tile
from concourse.bass_utils import run_bass_kernel_spmd

NCORES = 8
F32 = mybir.dt.float32
F32R = mybir.dt.float32r
BF16 = mybir.dt.bfloat16
FP8 = mybir.dt.float8e4
DR = mybir.MatmulPerfMode.DoubleRow


def _build_shards(char_lengths, L):
    """Index-only host prep: per-core word lists ((-1) = dummy), the shared
    schedule m_t, and per-core masks."""
    lengths = np.asarray(char_lengths)
    keep = np.where(lengths >= 2)[0]
    order = keep[np.argsort(-lengths[keep], kind="stable")]
    lens_sorted = lengths[order]

    core_pos = [[] for _ in range(NCORES)]
    core_mask = [[] for _ in range(NCORES)]
    profile = []
    idx = 0
    for l in range(L, 1, -1):
        c = int((lens_sorted == l).sum())
        if c == 0:
            continue
        n_l = (c + NCORES - 1) // NCORES
        words = order[idx:idx + c]
        idx += c
        for ci in range(NCORES):
            take = words[ci::NCORES]
            for w in take:
                core_pos[ci].append(int(w))
                core_mask[ci].append(1.0)
            for _ in range(n_l - len(take)):
                core_pos[ci].append(-1)
                core_mask[ci].append(0.0)
        profile.extend([l] * n_l)
    profile = np.array(profile)
    m_t = [int((profile > t).sum()) for t in range(L)]
    n = len(profile)
    return core_pos, core_mask, m_t, n


def _build_program(n, m_t, L, VC, DC, H, DW, HID, OUT, gwords, n_total, skip=()):
    H4 = 4 * H
    KH = H // 128                       # 4 h chunks
    GK = (128 * ((DW + 127) // 128)) // 128   # glove partition-major cols (3)
    SP = KH + GK                        # fused partial width (7)
    steps = [t for t in range(L) if m_t[t] > 0]
    KMLP = SP                           # head K-tiles

    nc = bacc.Bacc(num_devices=NCORES)

    oh_ext = nc.declare_dram_parameter("onehot", [len(steps), 128, n], BF16, isOutput=False)
    gath_ext = nc.declare_dram_parameter("gath", [128, GK, gwords], F32, isOutput=False)
    whh8_ext = nc.declare_dram_parameter("whh8", [128, KH, H4], FP8, isOutput=False)
    wih_ext = nc.declare_dram_parameter("wihT", [128, H4], BF16, isOutput=False)
    cemb_ext = nc.declare_dram_parameter("cembT", [128, VC], BF16, isOutput=False)
    bias_ext = nc.declare_dram_parameter("bias", [1, H4], BF16, isOutput=False)
    fc1_ext = nc.declare_dram_parameter("fc1T", [128, KMLP, HID], BF16, isOutput=False)
    fc1b_ext = nc.declare_dram_parameter("fc1b", [128, HID // 128], F32, isOutput=False)
    fc2_ext = nc.declare_dram_parameter("fc2T", [128, HID // 128, OUT], BF16, isOutput=False)
    fc2b_ext = nc.declare_dram_parameter("fc2b", [OUT], F32, isOutput=False)
    out_ext = nc.declare_dram_parameter("out", [1, OUT], F32, isOutput=True)

    sc_part = nc.dram_tensor("sc_part", [128 * KH], F32)
    sc_red = nc.dram_tensor("sc_red", [128 * KH], F32, addr_space="Shared")
    sg_part = nc.dram_tensor("sg_part", [128 * GK], F32)
    sg_red = nc.dram_tensor("sg_red", [128 * GK], F32, addr_space="Shared")
    du_part = nc.dram_tensor("du_part", [1], F32)
    du_red = nc.dram_tensor("du_red", [1], F32, addr_space="Shared")

    Sig = mybir.ActivationFunctionType.Sigmoid
    Tanh = mybir.ActivationFunctionType.Tanh
    Copy = mybir.ActivationFunctionType.Copy
    AX = mybir.AxisListType.X
    ADD = mybir.AluOpType.add
    MUL = mybir.AluOpType.mult

    with tile.TileContext(nc) as tc:
        with (
            tc.tile_pool(name="consts", bufs=1) as consts,
            tc.tile_pool(name="ohp", bufs=4) as ohp,
            tc.tile_pool(name="cell", bufs=2) as cell,
            tc.tile_pool(name="psg", bufs=2, space="PSUM") as psg,
        ):
            # dummy 1-float collective up front: absorbs the one-time global
            # collective BARRIER (~30us) under the LSTM instead of at the tail
            if "coll" not in skip:
                nc.gpsimd.collective_compute(
                    "AllReduce", ADD,
                    replica_groups=[list(range(NCORES))],
                    ins=[du_part[:]], outs=[du_red[:]],
                )

            # ---- critical-path constants (G feeds step 0) ----
            cemb_sb = consts.tile([128, VC], BF16, tag="cemb_sb")
            nc.sync.dma_start(out=cemb_sb, in_=cemb_ext[:, :])
            wih_sb = consts.tile([128, H4], BF16, tag="wih_sb")
            nc.sync.dma_start(out=wih_sb, in_=wih_ext[:, :])

            # Gaug = [char_embed @ W_ih^T ; bias row at VC; zeros above]
            g_bf = consts.tile([128, H4], BF16, tag="g_bf")
            nc.vector.memset(g_bf, 0.0)
            nc.sync.dma_start(out=g_bf[VC:VC + 1, :], in_=bias_ext[:, :])
            for c in range(H4 // 512):
                g_ps = psg.tile([128, 4, 512], F32, tag="ps")
                nc.tensor.matmul(
                    g_ps[:VC, 0, :],
                    cemb_sb,
                    wih_sb[:, c * 512:(c + 1) * 512],
                    start=True, stop=True,
                )
                nc.scalar.activation(g_bf[:VC, c * 512:(c + 1) * 512], g_ps[:VC, 0, :], Copy)

            # prefetch the first onehot slabs BEFORE the 1.6MB weight streams
            # so step 0 starts as soon as G is ready
            oh_pre = []
            for pi in range(2):
                ohp_t = ohp.tile([128, n], BF16, tag="oh", name=f"oh_pre{pi}")
                nc.sync.dma_start(out=ohp_t[:, :m_t[pi]], in_=oh_ext[pi, :, :m_t[pi]])
                oh_pre.append(ohp_t)
            whh8 = consts.tile([128, KH, H4], FP8, tag="whh8")
            nc.sync.dma_start(out=whh8, in_=whh8_ext[:, :, :])
            gath_sb = consts.tile([128, GK, gwords], F32, tag="gath_sb")
            nc.sync.dma_start(out=gath_sb, in_=gath_ext[:, :, :])

            one_sb = consts.tile([128, 1], F32, tag="one_sb")
            nc.vector.memset(one_sb, 1.0)

            # fused pooled-partial tile: [:, 0:KH] char sum, [:, KH:SP] glove
            sp_sb = consts.tile([128, SP], F32, tag="sp_sb")
            nc.vector.memset(sp_sb[:, 0:KH], 0.0)

            # ---- LSTM state, split per DR plane-pair so dependency tracking
            # stays precise: phase-0 matmuls wait only on h planes 0,1 ----
            hA = [consts.tile([128, 2, n], FP8, tag=f"hA{i}", name=f"hA{i}")
                  for i in range(2)]
            hB = [consts.tile([128, 2, n], FP8, tag=f"hB{i}", name=f"hB{i}")
                  for i in range(2)]
            cA = consts.tile([128, 2, n], BF16, tag="cA")
            cB = consts.tile([128, 2, n], BF16, tag="cB")

            def xmm(ps, oh_sb, j, m, stop):
                # one-hot (+bias row) matmuls: slot order g,i,f,o = gates 2,0,1,3
                for s, gate in enumerate((2, 0, 1, 3)):
                    mm = gate * KH + j
                    nc.tensor.matmul(
                        ps[:, s, :m],
                        g_bf[:, mm * 128:(mm + 1) * 128],
                        oh_sb[:, :m],
                        start=True, stop=stop,
                    )

            def rmm(ps, hp, m, j, stop):
                # fp8 DoubleRow recurrent matmuls over one h plane-pair tile
                p = 0 if hp in hA else 1
                for s, gate in enumerate((2, 0, 1, 3)):
                    mm = gate * KH + j
                    nc.tensor.matmul(
                        ps[:, s, :m],
                        whh8[:, 2 * p:2 * p + 2, mm * 128:(mm + 1) * 128],
                        hp[:, :, :m],
                        perf_mode=DR,
                        start=False, stop=stop,
                    )

            def acts(ps, ifo, gg, q, m, split=False):
                # gate activations for one chunk; sigmoid over i,f,o fused.
                # split=True releases i,f first so the cell chain (which only
                # needs i,f,g) starts ~m cycles earlier; o follows for h.
                nc.scalar.activation(gg[:, q, :m], ps[:, 0, :m], Tanh)
                if split:
                    nc.scalar.activation(ifo[:, q, 0:2, :m], ps[:, 1:3, :m], Sig)
                    nc.scalar.activation(ifo[:, q, 2, :m], ps[:, 3, :m], Sig)
                else:
                    nc.scalar.activation(ifo[:, q, :, :m], ps[:, 1:4, :m], Sig)

            def xmm2(ps, oh_sb, q, j, m, stop):
                # one-hot matmuls into slot q of a two-chunk pair tile.
                # PSUM start=True zeroes a whole 2KB bank; with two 1KB gate
                # slots per bank only the even slot may carry start, the odd
                # slot accumulates onto the pending-zeroed bytes.
                for s, gate in enumerate((2, 0, 1, 3)):
                    mm = gate * KH + j
                    nc.tensor.matmul(
                        ps[:, q, s, :m],
                        g_bf[:, mm * 128:(mm + 1) * 128],
                        oh_sb[:, :m],
                        start=(s % 2 == 0), stop=stop,
                        skip_group_check=True,
                    )

            def rmm2(ps, hp, m, q, j, stop):
                p = 0 if hp in hA else 1
                for s, gate in enumerate((2, 0, 1, 3)):
                    mm = gate * KH + j
                    nc.tensor.matmul(
                        ps[:, q, s, :m],
                        whh8[:, 2 * p:2 * p + 2, mm * 128:(mm + 1) * 128],
                        hp[:, :, :m],
                        perf_mode=DR,
                        start=False, stop=stop,
                        skip_group_check=True,
                    )

            def acts_pair(ps, ifo, gg, m):
                # gate activations for both chunks of a pair in 2 instructions
                nc.scalar.activation(gg[:, :, :m], ps[:, :, 0, :m], Tanh)
                nc.scalar.activation(ifo[:, :, :, :m], ps[:, :, 1:4, :m], Sig)

            def dve_c(ifo, gg, ig, cX, q, m, first):
                # c update for one chunk (slot q of its pair tiles)
                nc.vector.tensor_tensor(ig[:, q, :m], ifo[:, q, 0, :m],
                                        gg[:, q, :m], op=MUL)
                if first:
                    nc.vector.tensor_copy(cX[:, q, :m], ig[:, q, :m])
                else:
                    nc.vector.tensor_tensor(cX[:, q, :m], ifo[:, q, 1, :m],
                                            cX[:, q, :m], op=MUL)
                    nc.vector.tensor_tensor(cX[:, q, :m], cX[:, q, :m],
                                            ig[:, q, :m], op=ADD)

            def dve_h(ifo, tc_sb, cX, hX_wr, q, m):
                # tanh(c) then h = o * tanh(c), one chunk
                nc.scalar.activation(tc_sb[:, q, :m], cX[:, q, :m], Tanh)
                nc.vector.tensor_tensor(hX_wr[:, q, :m],
                                        ifo[:, q, 2, :m], tc_sb[:, q, :m], op=MUL)

            for si, t in enumerate(steps):
                m = m_t[t]
                hA_rd, hA_wr = hA[si % 2], hA[(si + 1) % 2]
                hB_rd, hB_wr = hB[si % 2], hB[(si + 1) % 2]
                if si < 2:
                    oh_sb = oh_pre[si]
                else:
                    oh_sb = ohp.tile([128, n], BF16, tag="oh")
                    nc.sync.dma_start(out=oh_sb[:, :m], in_=oh_ext[si, :, :m])

                ifoA = cell.tile([128, 2, 3, n], BF16, tag="ifoA")
                ggA = cell.tile([128, 2, n], BF16, tag="ggA")
                igA = cell.tile([128, 2, n], BF16, tag="igA")
                tcA = cell.tile([128, 2, n], BF16, tag="tcA")
                ifoB = cell.tile([128, 2, 3, n], BF16, tag="ifoB")
                ggB = cell.tile([128, 2, n], BF16, tag="ggB")
                igB = cell.tile([128, 2, n], BF16, tag="igB")
                tcB = cell.tile([128, 2, n], BF16, tag="tcB")

                first = si == 0
                if m > 256:
                    # PE order: x0 p00 x1 p01 p10 p11 x2 p02 x3 p03 p12 p13.
                    # x-MMs open each group (no fresh deps), phase-1 MMs are
                    # the stop legs; per-chunk cell chains start at each
                    # chunk's own sigmoid so pair A's h lands before the next
                    # step's p0 matmuls and pair B's before its p1 matmuls.
                    ps0 = psg.tile([128, 4, 512], F32, tag="ps", name="ps0")
                    xmm(ps0, oh_sb, 0, m, stop=first)
                    ps1 = psg.tile([128, 4, 512], F32, tag="ps", name="ps1")
                    xmm(ps1, oh_sb, 1, m, stop=first)
                    if not first:
                        rmm(ps0, hA_rd, m, 0, stop=False)
                        rmm(ps1, hA_rd, m, 1, stop=False)
                        rmm(ps0, hB_rd, m, 0, stop=True)
                    acts(ps0, ifoA, ggA, 0, m)
                    dve_c(ifoA, ggA, igA, cA, 0, m, first)
                    if not first:
                        rmm(ps1, hB_rd, m, 1, stop=True)
                    acts(ps1, ifoA, ggA, 1, m, split=True)
                    dve_c(ifoA, ggA, igA, cA, 1, m, first)
                    dve_h(ifoA, tcA, cA, hA_wr, 0, m)
                    dve_h(ifoA, tcA, cA, hA_wr, 1, m)
                    ps2 = psg.tile([128, 4, 512], F32, tag="ps", name="ps2")
                    xmm(ps2, oh_sb, 2, m, stop=first)
                    ps3 = psg.tile([128, 4, 512], F32, tag="ps", name="ps3")
                    xmm(ps3, oh_sb, 3, m, stop=first)
                    if not first:
                        rmm(ps2, hA_rd, m, 2, stop=False)
                        rmm(ps3, hA_rd, m, 3, stop=False)
                        rmm(ps2, hB_rd, m, 2, stop=True)
                    acts(ps2, ifoB, ggB, 0, m)
                    dve_c(ifoB, ggB, igB, cB, 0, m, first)
                    if not first:
                        rmm(ps3, hB_rd, m, 3, stop=True)
                    acts(ps3, ifoB, ggB, 1, m, split=True)
                    dve_c(ifoB, ggB, igB, cB, 1, m, first)
                    dve_h(ifoB, tcB, cB, hB_wr, 0, m)
                    dve_h(ifoB, tcB, cB, hB_wr, 1, m)
                else:
                    # small steps: both chunks of a pair share one PSUM tile
                    # [128, 8, 256] (slots q*4+s), halving ACT instruction
                    # count (tg/sig/tanh-c fire once per pair) and group count
                    psA = psg.tile([128, 2, 4, 256], F32, tag="ps", name="psA")
                    for q in (0, 1):
                        xmm2(psA, oh_sb, q, 0 * 2 + q, m, stop=first)
                    psB = psg.tile([128, 2, 4, 256], F32, tag="ps", name="psB")
                    for q in (0, 1):
                        xmm2(psB, oh_sb, q, 1 * 2 + q, m, stop=first)
                    if not first:
                        for q in (0, 1):
                            rmm2(psA, hA_rd, m, q, 0 * 2 + q, stop=False)
                        for q in (0, 1):
                            rmm2(psA, hB_rd, m, q, 0 * 2 + q, stop=True)
                    acts_pair(psA, ifoA, ggA, m)
                    dve_c(ifoA, ggA, igA, cA, 0, m, first)
                    dve_c(ifoA, ggA, igA, cA, 1, m, first)
                    dve_h(ifoA, tcA, cA, hA_wr, 0, m)
                    dve_h(ifoA, tcA, cA, hA_wr, 1, m)
                    if not first:
                        for q in (0, 1):
                            rmm2(psB, hA_rd, m, q, 1 * 2 + q, stop=False)
                        for q in (0, 1):
                            rmm2(psB, hB_rd, m, q, 1 * 2 + q, stop=True)
                    acts_pair(psB, ifoB, ggB, m)
                    dve_c(ifoB, ggB, igB, cB, 0, m, first)
                    dve_c(ifoB, ggB, igB, cB, 1, m, first)
                    dve_h(ifoB, tcB, cB, hB_wr, 0, m)
                    dve_h(ifoB, tcB, cB, hB_wr, 1, m)

                if si == 2:
                    # glove partial: one reduce over the gathered columns,
                    # then its AllReduce mid-LSTM (also re-syncs the cores so
                    # the tail char AllReduce runs at warm-collective speed)
                    nc.vector.tensor_reduce(sp_sb[:, KH:SP], gath_sb, axis=AX, op=ADD)
                    sg_pm = sg_part.rearrange("(p k) -> p k", k=GK)
                    nc.sync.dma_start(out=sg_pm, in_=sp_sb[:, KH:SP])
                    if "coll" in skip:
                        nc.sync.dma_start(out=sg_red[:], in_=sg_part[:])
                    else:
                        nc.gpsimd.collective_compute(
                            "AllReduce", ADD,
                            replica_groups=[list(range(NCORES))],
                            ins=[sg_part[:]], outs=[sg_red[:]],
                        )
                if si in (10, 16) and "coll" not in skip:
                    # keep the collective stack warm + cores synced so the
                    # tail AllReduce doesn't pay a cold/skew penalty; the DMA
                    # from this step's state anchors the collective in time
                    nc.sync.dma_start(out=du_part[:], in_=sp_sb[0:1, 0])
                    nc.gpsimd.collective_compute(
                        "AllReduce", ADD,
                        replica_groups=[list(range(NCORES))],
                        ins=[du_part[:]], outs=[du_red[:]],
                    )

                next_m = m_t[steps[si + 1]] if si + 1 < len(steps) else 0
                if next_m < m:
                    rt = cell.tile([128, KH], F32, tag="rt")
                    nc.vector.tensor_reduce(rt[:, 0:2], hA_wr[:, :, next_m:m], axis=AX, op=ADD)
                    nc.vector.tensor_reduce(rt[:, 2:4], hB_wr[:, :, next_m:m], axis=AX, op=ADD)
                    nc.vector.tensor_tensor(sp_sb[:, 0:KH], sp_sb[:, 0:KH], rt, op=ADD)

                if si == len(steps) - 6:
                    # head weights mid-kernel so the DMA queue is clear early
                    fc1_sb = consts.tile([128, KMLP, HID], BF16, tag="fc1_sb")
                    nc.sync.dma_start(out=fc1_sb, in_=fc1_ext[:, :, :])
                    fc1b_sb = consts.tile([128, HID // 128], F32, tag="fc1b_sb")
                    nc.sync.dma_start(out=fc1b_sb, in_=fc1b_ext[:, :])
                    fc2_sb = consts.tile([128, HID // 128, OUT], BF16, tag="fc2_sb")
                    nc.sync.dma_start(out=fc2_sb, in_=fc2_ext[:, :, :])
                    fc2b_sb = consts.tile([128, 1], F32, tag="fc2b_sb")
                    nc.sync.dma_start(out=fc2b_sb[:OUT, 0], in_=fc2b_ext[:])

            # ---- char partial -> tail AllReduce ----
            sc_pm = sc_part.rearrange("(p k) -> p k", k=KH)
            nc.sync.dma_start(out=sc_pm, in_=sp_sb[:, 0:KH])
            if "coll" in skip:
                nc.sync.dma_start(out=sc_red[:], in_=sc_part[:])
            else:
                nc.gpsimd.collective_compute(
                    "AllReduce", ADD,
                    replica_groups=[list(range(NCORES))],
                    ins=[sc_part[:]], outs=[sc_red[:]],
                )

            # ---- head MLP (identical on every core; bf16 matmuls) ----
            avg_f = consts.tile([128, SP], F32, tag="avg_f")
            nc.sync.dma_start(out=avg_f[:, 0:KH],
                              in_=sc_red.rearrange("(p k) -> p k", k=KH))
            nc.sync.dma_start(out=avg_f[:, KH:SP],
                              in_=sg_red.rearrange("(p k) -> p k", k=GK))
            avg_sb = consts.tile([128, SP], BF16, tag="avg_sb")
            nc.vector.tensor_copy(avg_sb, avg_f)
            # fc1 preacts directly partition-major: accumulate K=1 matvecs
            # per 128-wide hidden chunk (no transpose pass needed)
            pc_ps = psg.tile([128, 4, 512], F32, tag="ps", name="pc_ps")
            for i in range(HID // 128):
                for k in range(KMLP):
                    nc.tensor.matmul(pc_ps[:, 0, i:i + 1],
                                     fc1_sb[:, k, i * 128:(i + 1) * 128],
                                     avg_sb[:, k:k + 1],
                                     start=(k == 0), stop=(k == KMLP - 1))
            h1_sb = consts.tile([128, HID // 128], BF16, tag="h1_sb")
            for i in range(HID // 128):
                nc.scalar.activation(h1_sb[:, i:i + 1], pc_ps[:, 0, i:i + 1], Sig,
                                     bias=fc1b_sb[:, i:i + 1], scale=1.0 / n_total)
            lo_sb = consts.tile([128, 1], F32, tag="lo_sb")
            lp = psg.tile([128, 4, 512], F32, tag="ps", name="lp")
            for k in range(HID // 128):
                nc.tensor.matmul(lp[:OUT, 0, 0:1], fc2_sb[:, k, :], h1_sb[:, k:k + 1],
                                 start=(k == 0), stop=(k == HID // 128 - 1))
            nc.vector.tensor_tensor(lo_sb[:OUT, :], lp[:OUT, 0, 0:1], fc2b_sb[:OUT, :], op=ADD)
            nc.sync.dma_start(out=out_ext[0, :], in_=lo_sb[:OUT, 0])

    nc.compile()
    return nc


def kernel(**inputs):
    word_indices = np.asarray(inputs["word_indices"])
    char_indices = np.asarray(inputs["char_indices"])
    char_lengths = np.asarray(inputs["char_lengths"])
    glove_table = np.ascontiguousarray(np.asarray(inputs["glove_table"], dtype=np.float32))
    char_embed = np.asarray(inputs["char_embed"], dtype=np.float32)
    W_ih = np.asarray(inputs["W_ih"], dtype=np.float32)
    W_hh = np.asarray(inputs["W_hh"], dtype=np.float32)
    b_ih = np.asarray(inputs["b_ih"], dtype=np.float32)
    b_hh = np.asarray(inputs["b_hh"], dtype=np.float32)
    fc1_W = np.asarray(inputs["fc1_W"], dtype=np.float32)
    fc1_b = np.asarray(inputs["fc1_b"], dtype=np.float32)
    fc2_W = np.asarray(inputs["fc2_W"], dtype=np.float32)
    fc2_b = np.asarray(inputs["fc2_b"], dtype=np.float32)

    N, L = char_indices.shape
    VW, DW = glove_table.shape
    VC, DC = char_embed.shape
    H = W_hh.shape[1]
    H4 = 4 * H
    HID = fc1_W.shape[0]
    OUT = fc2_W.shape[0]
    KH = H // 128

    core_pos, core_mask, m_t, n = _build_shards(char_lengths, L)
    assert n <= 512, f"per-core shard {n} exceeds one PSUM bank"
    steps = [t for t in range(L) if m_t[t] > 0]
    gwords = N // NCORES

    nc = _build_program(n, m_t, L, VC, DC, H, DW, HID, OUT, gwords, N)

    GK = (128 * ((DW + 127) // 128)) // 128
    SP = KH + GK
    # fc1 rows: char block first [H], then glove block [DW], padded to 128*SP
    fc1T = np.zeros((128 * SP, HID), np.float32)
    fc1T[:H] = fc1_W[:, DW:].T
    fc1T[H:H + DW] = fc1_W[:, :DW].T
    shared = dict(
        whh8=np.ascontiguousarray(
            W_hh.T.reshape(KH, 128, H4).transpose(1, 0, 2)).astype(ml_dtypes.float8_e4m3fn),
        wihT=np.ascontiguousarray(W_ih.T).astype(ml_dtypes.bfloat16),
        cembT=np.ascontiguousarray(char_embed.T).astype(ml_dtypes.bfloat16),
        bias=(b_ih + b_hh).reshape(1, H4).astype(ml_dtypes.bfloat16),
        fc1T=np.ascontiguousarray(
            fc1T.reshape(SP, 128, HID).transpose(1, 0, 2)).astype(ml_dtypes.bfloat16),
        fc1b=np.ascontiguousarray(fc1_b.reshape(-1, 128).T),
        fc2T=np.ascontiguousarray(
            fc2_W.T.reshape(-1, 128, OUT).transpose(1, 0, 2)).astype(ml_dtypes.bfloat16),
        fc2b=fc2_b,
    )

    in_maps = []
    for ci in range(NCORES):
        pos = core_pos[ci]
        mask = np.array(core_mask[ci], np.float32)
        # one-hot char encodings [steps, 128, n] with bias row VC = validity
        oh = np.zeros((len(steps), 128, n), ml_dtypes.bfloat16)
        widx = np.array([w if w >= 0 else 0 for w in pos])
        ci_shard = char_indices[widx]                    # [n, L]
        real = mask > 0
        cols = np.arange(n)
        for si, t in enumerate(steps):
            oh[si, ci_shard[real, t], cols[real]] = 1.0
            oh[si, VC, real] = 1.0
        # gathered glove rows, transposed partition-major [128, GK, gwords]
        words = word_indices[ci * gwords:(ci + 1) * gwords]
        g = np.zeros((128 * GK, gwords), np.float32)
        g[:DW] = glove_table[words].T
        in_maps.append(dict(
            onehot=oh,
            gath=np.ascontiguousarray(g.reshape(GK, 128, gwords).transpose(1, 0, 2)),
            **shared,
        ))

    # the axon/NRT stack occasionally reports a transient device error
    # (NRT_EXEC_UNIT_UNRECOVERABLE); a retry on fresh state recovers it
    res = None
    for attempt in range(3):
        try:
            res = run_bass_kernel_spmd(nc, in_maps, list(range(NCORES)))
            break
        except Exception:
            if attempt == 2:
                raise
            time.sleep(2.0)
    global _LAST_RESULTS
    _LAST_RESULTS = res
    return np.array(res.results[0]["out"], dtype=np.float32)


_LAST_RESULTS = None
